# revision 1
# baseline (speedup 1.0000x reference)
"""Trainium2 Bass kernel for nn_BlockLinear_MixerBlock (6-layer radix-4 butterfly mixer).

Math: the reference applies 6 block-diagonal butterfly layers (radix 4, gaps
1,4,16,64,256,1024) along the feature dim (4096) of x [8192, 4096].
Layers 0-2 compose into a dense 64x64 mix within each contiguous 64-chunk
(stage A); layers 3-5 compose into a dense 64x64 mix across stride-64
feature classes (stage B) — a Monarch factorization:

    y = PermOut( B_blockdiag @ Perm( A_blockdiag @ x^T ) )

Device dataflow per 128-row batch tile (per core, data-parallel over 8 cores):
  load: gpsimd (SWDGE) casting DMA reads x f32 from HBM, writes f16 SBUF
  T1: PE-transpose f16 x slices (1 cycle/row; quads of 4 per PSUM bank),
      one eviction copy per quad -> xT f16 in SBUF
  A : matmul(lhsT=xT_t f16, rhs=WA[:, 128t:128t+128] f16) -> [128b, 128] f32
      (data-stationary: output lands b-major; f16 runs 1 cycle/row at N=128)
      grouped eviction scatters 4 outputs at once into u-major y1 (f32r)
  T3: PE-transpose contiguous 128-col slices of u-major y1 -> z (cast f16)
  B : matmul(lhsT=z_m f16, rhs=WB[:, 128m:128m+128] f16) -> [128b, 128]
      grouped scatter-eviction into natural output layout, DMA out (f32).
Software-pipelined at tile level (tile i load+A before tile i-1 B);
PSUM evictions alternate DVE/ACT. x is f16 on-chip only; HBM I/O stays f32.

WA/WB are built on the host from `weights` by composing the exact reference
layers against the identity (float64), stored f16 (adds ~2^-11 relative
error on top of the PE's fp22 internal truncation; measured 4.9e-4 total).
"""

import numpy as np

import concourse.bass as bass
import concourse.bacc as bacc
import concourse.mybir as mybir
from concourse.tile import TileContext
from concourse.bass_utils import run_bass_kernel_spmd

# ---- problem constants (hardcoded per contract) ----
N_CORES = 8
BS = 8192
D = 4096
BD = 4
NUM_LAYERS = 6
GAPS = [1, 4, 16, 64, 256, 1024]
BPC = BS // N_CORES          # 1024 batch rows per core
NBT = BPC // 128             # 8 batch tiles per core
NFT = D // 128               # 32 feature tiles

F32 = mybir.dt.float32
F32R = mybir.dt.float32r
F16 = mybir.dt.float16


# ---------------- host-side weight composition ----------------

def _ref_layers(x, weights, layers):
    bs = x.shape[0]
    y = x
    for i in layers:
        gap = GAPS[i]
        y = y.reshape(bs, -1, BD, gap).swapaxes(2, 3)
        y = y.reshape(bs, -1, BD)
        y = np.einsum('bnk,nkm->bnm', y, weights[i])
        y = y.reshape(bs, -1, gap, BD).swapaxes(2, 3)
    return y.reshape(bs, -1)


def _build_stage_weights(weights):
    w64 = weights.astype(np.float64)
    I = np.eye(D, dtype=np.float64)
    MA = _ref_layers(I, w64, [0, 1, 2])   # y1 = x @ MA (block-diag, 64-chunks)
    MB = _ref_layers(I, w64, [3, 4, 5])   # y  = y1 @ MB (block over stride-64)

    WA = np.zeros((128, D), np.float16)
    MA32 = MA.astype(np.float16)
    for t in range(NFT):
        WA[:, 128 * t:128 * (t + 1)] = MA32[128 * t:128 * (t + 1), 128 * t:128 * (t + 1)]

    WB = np.zeros((128, D), np.float16)
    MBr = MB.astype(np.float16).reshape(64, 64, 64, 64)   # [c, u, c', u']
    for m in range(NFT):
        blk = MBr[:, 2 * m:2 * m + 2, :, 2 * m:2 * m + 2]  # [c, d, c', d']
        WB[:, 128 * m:128 * (m + 1)] = blk.transpose(1, 0, 3, 2).reshape(128, 128)
    return WA, WB


# ---------------- device program ----------------

def _build_program(repeats=1, timing_io=False):
    nc = bacc.Bacc("TRN2", target_bir_lowering=False, debug=False)
    if timing_io:
        # timing-only variant: big tensors live in device DRAM (no host I/O),
        # so per-call wall time is not dominated by 256 MiB of transfers
        x_d = nc.dram_tensor("x_int", [BPC, D], F32R, kind="Internal")
        y_d = nc.dram_tensor("y_int", [BPC, D], F32, kind="Internal")
        yp_d = nc.dram_tensor("yprobe", [128, 4], F32, kind="ExternalOutput")
    else:
        x_d = nc.dram_tensor("x", [BPC, D], F32R, kind="ExternalInput")
        y_d = nc.dram_tensor("y", [BPC, D], F32, kind="ExternalOutput")
        yp_d = None
    wa_d = nc.dram_tensor("wa", [128, D], F16, kind="ExternalInput")
    wb_d = nc.dram_tensor("wb", [128, D], F16, kind="ExternalInput")
    id_d = nc.dram_tensor("ident", [128, 128], F32R, kind="ExternalInput")

    def copy_engine(k):
        # alternate PSUM-eviction copies between DVE and ACT
        return nc.vector if k % 2 == 0 else nc.scalar

    def do_copy(eng, out_ap, in_ap):
        if eng is nc.vector:
            eng.tensor_copy(out_ap, in_ap)
        else:
            eng.copy(out_ap, in_ap)

    with TileContext(nc) as tc:
        with (
            tc.tile_pool(name="const", bufs=1) as const,
            tc.tile_pool(name="xin", bufs=2) as xin_pool,
            tc.tile_pool(name="mid", bufs=2) as mid_pool,
            tc.tile_pool(name="outp", bufs=2) as out_pool,
            tc.tile_pool(name="small", bufs=4) as small_pool,
            tc.tile_pool(name="psT", bufs=3, space="PSUM") as psT_pool,
            tc.tile_pool(name="psM", bufs=4, space="PSUM") as psM_pool,
        ):
            ident = const.tile([128, 128], F32R, name="ident_sb")
            nc.sync.dma_start(ident[:], id_d.ap())
            ident_r = ident[:]
            ident16 = const.tile([128, 128], F16, name="ident16_sb")
            nc.vector.tensor_copy(ident16[:], ident[:])
            # prefetch batch-tile 0 of x BEFORE the (large) weight loads so the
            # first transposes are not stuck behind 4 MiB of weight DMA
            x0 = None
            if repeats == 1 and not timing_io:
                x0 = xin_pool.tile([128, D], F16, name="x_nat", tag="x_nat")
                for q in range(8):
                    nc.gpsimd.dma_start(
                        x0[:, 512 * q:512 * (q + 1)],
                        x_d.ap()[0:128, 512 * q:512 * (q + 1)],
                    )
            wa_sb = const.tile([128, D], F16, name="wa_sb")
            wb_sb = const.tile([128, D], F16, name="wb_sb")
            for h in range(8):
                lo = 512 * h
                hi = 512 * (h + 1)
                nc.sync.dma_start(wa_sb[:, lo:hi], wa_d.ap()[:, lo:hi])
                nc.sync.dma_start(wb_sb[:, lo:hi], wb_d.ap()[:, lo:hi])

            import contextlib
            rep_ctx = tc.For_i(0, repeats, 1) if repeats > 1 else contextlib.nullcontext()
            with rep_ctx:
                _body(nc, tc, x_d, y_d, wa_sb, wb_sb, ident_r, ident16,
                      xin_pool, mid_pool, out_pool, small_pool, psT_pool, psM_pool,
                      x0=x0)
            if yp_d is not None:
                probe = const.tile([128, 4], F32, name="probe_sb")
                nc.sync.dma_start(probe[:], y_d.ap()[0:128, 0:4])
                nc.sync.dma_start(yp_d.ap()[:, :], probe[:])
    nc.compile()
    return nc


def _body(nc, tc, x_d, y_d, wa_sb, wb_sb, ident_r, ident16,
          xin_pool, mid_pool, out_pool, small_pool, psT_pool, psM_pool,
          x0=None):
    F32 = mybir.dt.float32
    F32R = mybir.dt.float32r
    F16 = mybir.dt.float16

    def copy_engine(k):
        return nc.vector if k % 2 == 0 else nc.scalar

    def do_copy(eng, out_ap, in_ap):
        if eng is nc.vector:
            eng.tensor_copy(out_ap, in_ap)
        else:
            eng.copy(out_ap, in_ap)

    def emit_load_and_A(i):
        if i == 0 and x0 is not None:
            x_nat = x0
        else:
            x_nat = xin_pool.tile([128, D], F16, name="x_nat", tag="x_nat")
            for q in range(8):
                nc.gpsimd.dma_start(
                    x_nat[:, 512 * q:512 * (q + 1)],
                    x_d.ap()[128 * i:128 * (i + 1), 512 * q:512 * (q + 1)],
                )
        # stage-A output stored u-major: column 64*u + c holds feature 64*c + u
        y1u = mid_pool.tile([128, D], F32R, name="y1u", tag="y1u")
        # grouped scatter: dst col 64*u + c with c = 8q + 2*jj + c2
        y1g = y1u[:].rearrange("b (u c) -> b c u", c=64)

        def a_tquad(q):
            psT = psT_pool.tile([128, 512], F16, name="psT", tag="psT")
            for j in range(4):
                t = 4 * q + j
                nc.tensor.transpose(
                    psT[:, 128 * j:128 * (j + 1)],
                    x_nat[:, 128 * t:128 * (t + 1)],
                    ident16[:],
                )
            xt = small_pool.tile([128, 512], F16, name="xt", tag="xt")
            do_copy(copy_engine(q), xt[:], psT[:])
            return xt

        def a_mmgroup(q, xt):
            psA = psM_pool.tile([128, 512], F32, name="psA", tag="psAB")
            for j in range(4):
                t = 4 * q + j
                nc.tensor.matmul(
                    psA[:, 128 * j:128 * (j + 1)],
                    lhsT=xt[:, 128 * j:128 * (j + 1)],
                    rhs=wa_sb[:, 128 * t:128 * (t + 1)],
                    start=True, stop=True,
                )
            # grouped eviction: cols = (jj:4 step 128, c2:2 step 64, u:64 step 1)
            src = psA[:].rearrange("b (jj c2 u) -> b jj c2 u", jj=4, c2=2)
            dst = y1g[:, 8 * q:8 * q + 8, :].rearrange("b (jj c2) u -> b jj c2 u", jj=4)
            do_copy(copy_engine(q + 1), dst, src)

        # 1-quad skew: transposes run one group ahead of the matmuls that
        # consume them, so PE never stalls on the PSUM->SBUF copy
        xt_prev = None
        for q in range(8):
            xt = a_tquad(q)
            if xt_prev is not None:
                a_mmgroup(q - 1, xt_prev)
            xt_prev = xt
        a_mmgroup(7, xt_prev)
        return y1u

    def emit_B(i, y1u):
        yob = out_pool.tile([128, D], F32, name="yob", tag="yob")
        # dst col 64*c + 2*m + d with m = 4q + jj
        yog = yob[:].rearrange("b (c u) -> b u c", u=64)

        def b_tquad(q):
            psT3 = psT_pool.tile([128, 512], F32R, name="psT3", tag="psT")
            for j in range(4):
                m = 4 * q + j
                nc.tensor.transpose(
                    psT3[:, 128 * j:128 * (j + 1)],
                    y1u[:, 128 * m:128 * (m + 1)],
                    ident_r,
                )
            z = small_pool.tile([128, 512], F16, name="z", tag="z")
            do_copy(copy_engine(q + 1), z[:], psT3[:])
            return z

        def b_mmgroup(q, z):
            psB = psM_pool.tile([128, 512], F32, name="psB", tag="psAB")
            for j in range(4):
                m = 4 * q + j
                nc.tensor.matmul(
                    psB[:, 128 * j:128 * (j + 1)],
                    lhsT=z[:, 128 * j:128 * (j + 1)],
                    rhs=wb_sb[:, 128 * m:128 * (m + 1)],
                    start=True, stop=True,
                )
            src = psB[:].rearrange("b (jj d c) -> b jj d c", jj=4, d=2)
            dst = yog[:, 8 * q:8 * q + 8, :].rearrange("b (jj d) c -> b jj d c", jj=4)
            do_copy(copy_engine(q), dst, src)

        z_prev = None
        for q in range(8):
            z = b_tquad(q)
            if z_prev is not None:
                b_mmgroup(q - 1, z_prev)
            z_prev = z
        b_mmgroup(7, z_prev)
        nc.sync.dma_start(y_d.ap()[128 * i:128 * (i + 1), :], yob[:])

    # 1-deep software pipeline: emit tile i's load+A before tile (i-1)'s B so
    # the PE stream has stage-A work to chew on while stage-B inputs settle
    y1_prev = None
    for i in range(NBT):
        y1 = emit_load_and_A(i)
        if y1_prev is not None:
            emit_B(i - 1, y1_prev)
        y1_prev = y1
    emit_B(NBT - 1, y1_prev)


_PROGRAMS = {}


def _get_program(repeats=1):
    if repeats not in _PROGRAMS:
        _PROGRAMS[repeats] = _build_program(repeats)
    return _PROGRAMS[repeats]


def _run(x, weights, repeats=1, **spmd_kwargs):
    assert x.shape == (BS, D), x.shape
    WA, WB = _build_stage_weights(np.asarray(weights, dtype=np.float32))
    ident = np.eye(128, dtype=np.float32)
    x = np.ascontiguousarray(np.asarray(x, dtype=np.float32))
    nc = _get_program(repeats)
    in_maps = [
        {
            "x": x[c * BPC:(c + 1) * BPC, :],
            "wa": WA,
            "wb": WB,
            "ident": ident,
        }
        for c in range(N_CORES)
    ]
    res = run_bass_kernel_spmd(nc, in_maps, core_ids=list(range(N_CORES)), **spmd_kwargs)
    y = np.concatenate([res.results[c]["y"] for c in range(N_CORES)], axis=0)
    return y.astype(np.float32, copy=False), res


def kernel(x, weights):
    y, _ = _run(x, weights)
    return y


def _run_timing(weights, repeats, n_calls=6):
    """Delta-timing helper: runs the internal-I/O variant; returns wall times."""
    import time
    WA, WB = _build_stage_weights(np.asarray(weights, dtype=np.float32))
    ident = np.eye(128, dtype=np.float32)
    key = ("timing", repeats)
    if key not in _PROGRAMS:
        _PROGRAMS[key] = _build_program(repeats, timing_io=True)
    nc = _PROGRAMS[key]
    in_maps = [{"wa": WA, "wb": WB, "ident": ident} for _ in range(N_CORES)]
    walls = []
    for _ in range(n_calls):
        t0 = time.time()
        run_bass_kernel_spmd(nc, in_maps, core_ids=list(range(N_CORES)))
        walls.append(time.time() - t0)
    return walls

